# revision 1
# baseline (speedup 1.0000x reference)
"""Self-contained Trainium2 Bass kernel for the SLAYER SNN problem.

kernel(**inputs) takes FULL inputs {spikeInput:[64,4,2000], W1:[512,4],
W2:[2,512]} and returns the FULL [64,2,2000] output. Batch is sharded
8-ways across NeuronCores; each core runs an identical program on its
8 samples.
"""
from contextlib import ExitStack

import numpy as np

import concourse.bass as bass
import concourse.mybir as mybir
from concourse.bass_utils import run_bass_kernel_spmd
from concourse.tile import TileContext
import concourse.tile as _tile_mod
from concourse.vector_clock import ScopedClock as _ScopedClock, VectorClock as _VectorClock


def _drain_and_barrier_split(self, tick_clock, wait_clock):
    # Workaround for walrus "Too many sync wait commands" on the Tile tail
    # drain: emit one drain per processor instead of one multi-wait drain.
    gc = tick_clock.global_clock
    ticks = list(gc)
    for p, t in enumerate(ticks):
        if t <= 0:
            continue
        sub = [t if q == p else 0 for q in range(len(ticks))]
        drain_inst = self.nc.sync.drain()
        wait_clock.add_sem_waits(
            drain_inst.ins, _ScopedClock({None: _VectorClock(sub)}))
    self.nc.all_engine_barrier()
    assert self.sems is not None
    popped = self.nc._tile_sem_poison_stack.pop()
    assert popped is self._sem_poison
    self.nc.clear_and_free_semaphores(list(self.sems.allocated().values()))
    self.nc.all_engine_barrier()


_tile_mod.TileContext._drain_and_barrier = _drain_and_barrier_split


def _split_waits_json(raw):
    # walrus in this container accepts at most one sem-wait per instruction;
    # spill extras onto same-engine Drain carriers placed just before.
    import json as _json
    m = _json.loads(raw)
    ctr = 0
    for fn in m["functions"]:
        for bb in fn["blocks"]:
            out = []
            for i in bb.get("instructions", []):
                si = i.get("sync_info") or {}
                w = si.get("on_wait") or []
                if len(w) > 1:
                    for chunk in w[:-1]:
                        ctr += 1
                        out.append({
                            "debug": i.get("debug", 0), "engine": i["engine"],
                            "ins": [], "name": f"I-WS{ctr}", "opcode": "Drain",
                            "outs": [], "sync_info": {"on_wait": [chunk]},
                        })
                    si = dict(si)
                    si["on_wait"] = w[-1:]
                    i = dict(i)
                    i["sync_info"] = si
                out.append(i)
            bb["instructions"] = out
    return _json.dumps(m).encode()


def _install_wait_split(nc):
    orig = nc.to_json_bytes
    nc.to_json_bytes = lambda: _split_waits_json(orig())
    return nc

F32 = mybir.dt.float32
BF16 = mybir.dt.bfloat16
ALU = mybir.AluOpType

DS = float(np.exp(np.float32(-1.0 / 10.0), dtype=np.float32))
DR = float(np.exp(np.float32(-1.0 / 1.0), dtype=np.float32))
CS = float(np.float32(np.e / 10.0))
CR = float(np.float32(-2.0 * 10.0 * np.e / 1.0))
TH = 10.0

B = 64
N_CORES = 8
B_LOC = 8
NIN = 4
H = 512
HC = 4
NOUT = 2
NJ = 34
PAD_B = 128
LAG = 2
T_FULL = 2000
L_BLK = 125

_nc_cache = {}


def build(T: int = T_FULL, L: int = L_BLK):
    NB = T // L
    assert NB * L == T and L <= 125 and NB >= LAG
    nc = bass.Bass("TRN2", target_bir_lowering=False, debug=False, num_devices=N_CORES)

    x_in = nc.declare_dram_parameter("x", [NIN, B_LOC, T], F32, isOutput=False)
    w1t_in = nc.declare_dram_parameter("w1t", [NIN, H], F32, isOutput=False)
    w2t_in = nc.declare_dram_parameter("w2t", [128, HC * NOUT], F32, isOutput=False)
    out_d = nc.declare_dram_parameter("out", [B_LOC * NOUT, T], F32, isOutput=True)

    with TileContext(nc) as tc, ExitStack() as ctx:
        pool = ctx.enter_context(tc.tile_pool(name="main", bufs=1))
        psum = ctx.enter_context(tc.tile_pool(name="ps", bufs=1, space="PSUM"))

        w1t = pool.tile([NIN, H], F32, tag="w1t", name="w1t")
        nc.sync.dma_start(out=w1t[:], in_=w1t_in[:])
        w2t = pool.tile([128, HC * NOUT], F32, tag="w2t", name="w2t")
        nc.sync.dma_start(out=w2t[:], in_=w2t_in[:])
        dsc = pool.tile([128, L], F32, tag="dsc", name="dsc")
        nc.vector.memset(dsc[:], DS)

        def zeros(shape, tag, dtype=F32, eng=None):
            t = pool.tile(shape, dtype, tag=tag, name=tag)
            (eng or nc.vector).memset(t[:], 0.0)
            return t

        yr = zeros([128, NJ], "yr")
        xr = zeros([128, NJ], "xr")
        u_t = pool.tile([128, NJ], F32, tag="u", name="u")
        w_t = pool.tile([128, NJ], F32, tag="w", name="w")

        P_blk = [zeros([128, NJ * L], f"P{i}") for i in range(2)]
        S_blk = [zeros([128, NJ * L], f"S{i}") for i in range(2)]

        xs_seg = [zeros([128, L], f"xs{j}") for j in range(32)]
        ys_seg = [zeros([128, L + 1], f"ys{j}") for j in range(32)]

        x2_seg = zeros([16, L], "x2")
        y2_seg = zeros([16, L + 1], "y2")
        a2_ps = [psum.tile([2, B_LOC * PAD_B], F32, tag=f"a2ps{i}", name=f"a2ps{i}")
                 for i in range(2)]
        a2_sb = [pool.tile([2, B_LOC * L], F32, tag=f"a2sb{i}", name=f"a2sb{i}")
                 for i in range(2)]
        a2_16 = [pool.tile([16, L], F32, tag=f"a216{i}", name=f"a216{i}")
                 for i in range(2)]
        x3 = zeros([16, L], "x3")
        y3 = zeros([16, L + 1], "y3")
        o3 = pool.tile([16, L], F32, tag="o3", name="o3")
        a1_ps = [psum.tile([128, L], F32, tag=f"a1ps{i}", name=f"a1ps{i}")
                 for i in range(3)]
        a1_sb = [pool.tile([128, L], F32, tag=f"a1sb{i}", name=f"a1sb{i}")
                 for i in range(3)]

        xsb = [pool.tile([NIN, B_LOC * L], F32, tag=f"xsb{i}", name=f"xsb{i}")
               for i in range(2)]

        xv = x_in[:]

        def produce_L1(k):
            Pb = P_blk[k % 2]
            t0 = k * L
            xb = xsb[k % 2]
            nc.sync.dma_start(
                out=xb[:].rearrange("i (b t) -> i b t", t=L),
                in_=xv[:, :, t0:t0 + L])
            xbv = xb[:].rearrange("i (b t) -> i b t", t=L)
            for j in range(32):
                hc, b = divmod(j, B_LOC)
                aps, asb = a1_ps[j % 3], a1_sb[j % 3]
                nc.tensor.matmul(
                    aps[:],
                    lhsT=w1t[:, hc * 128:(hc + 1) * 128],
                    rhs=xbv[:, b, :],
                    start=True, stop=True)
                nc.scalar.copy(asb[:], aps[:])
                nc.vector.tensor_tensor_scan(
                    xs_seg[j][:], dsc[:], asb[:],
                    initial=xs_seg[j][:, L - 1:L], op0=ALU.mult, op1=ALU.add)
                nc.scalar.mul(Pb[:, j * L:j * L + 1], ys_seg[j][:, L:L + 1], CS)
                nc.vector.tensor_tensor_scan(
                    ys_seg[j][:, 1:L + 1], xs_seg[j][:], dsc[:],
                    initial=ys_seg[j][:, L:L + 1], op0=ALU.add, op1=ALU.mult)
                nc.scalar.mul(Pb[:, j * L + 1:(j + 1) * L], ys_seg[j][:, 1:L], CS)

        def produce_L2(k):
            # P for L2-time block k-LAG into P_blk col 32 (partitions 0..15,
            # p = o*8+b)
            Pb = P_blk[k % 2]
            asb2 = a2_sb[k % 2]
            a16 = a2_16[k % 2]
            nc.scalar.copy(
                asb2[:].rearrange("o (b t) -> o b t", t=L),
                a2_ps[(k - LAG) % 2][:].rearrange("o (b t) -> o b t", t=PAD_B)[:, :, 0:L])
            for b in range(B_LOC):
                # pack p = 2*b + o (DMA partition shift; contiguous 2-row dst)
                nc.sync.dma_start(out=a16[2 * b:2 * b + 2, :],
                                  in_=asb2[:, b * L:(b + 1) * L])
            nc.vector.tensor_tensor_scan(
                x2_seg[:], dsc[0:16, :], a16[:],
                initial=x2_seg[:, L - 1:L], op0=ALU.mult, op1=ALU.add)
            nc.scalar.mul(Pb[0:16, 32 * L:32 * L + 1], y2_seg[:, L:L + 1], CS)
            nc.vector.tensor_tensor_scan(
                y2_seg[:, 1:L + 1], x2_seg[:], dsc[0:16, :],
                initial=y2_seg[:, L:L + 1], op0=ALU.add, op1=ALU.mult)
            nc.scalar.mul(Pb[0:16, 32 * L + 1:33 * L], y2_seg[:, 1:L], CS)

        def step_block(k, narrow=False):
            Pb, Sb = P_blk[k % 2], S_blk[k % 2]
            for tau in range(L):
                if narrow:
                    # only the L2 column is live (tail blocks)
                    Pcol = Pb[0:16, 32 * L + tau:32 * L + tau + 1]
                    Scol = Sb[0:16, 32 * L + tau:32 * L + tau + 1]
                    uu, yy, xx, ww = (u_t[0:16, 32:33], yr[0:16, 32:33],
                                      xr[0:16, 32:33], w_t[0:16, 32:33])
                else:
                    Pcol, Scol = Pb[:, tau::L], Sb[:, tau::L]
                    uu, yy, xx, ww = u_t[:], yr[:], xr[:], w_t[:]
                nc.vector.scalar_tensor_tensor(uu, yy, CR, Pcol, ALU.mult, ALU.add)
                nc.vector.tensor_scalar(Scol, uu, TH, None, ALU.is_ge)
                nc.vector.scalar_tensor_tensor(xx, xx, DR, Scol, ALU.mult, ALU.add)
                nc.vector.tensor_tensor(out=ww, in0=yy, in1=xx, op=ALU.add)
                nc.vector.tensor_scalar(yy, ww, DR, None, ALU.mult)

        def post_block(k):
            Sb = S_blk[k % 2]
            NBASES = T // L
            if k < NBASES:
                for b in range(B_LOC):
                    for hc in range(HC):
                        j = hc * B_LOC + b
                        nc.tensor.matmul(
                            a2_ps[k % 2][:, b * PAD_B:b * PAD_B + L],
                            lhsT=w2t[:, hc * NOUT:(hc + 1) * NOUT],
                            rhs=Sb[:, j * L:(j + 1) * L],
                            start=(hc == 0), stop=(hc == HC - 1))
            if k >= LAG:
                kk = k - LAG
                s2seg = Sb[0:16, 32 * L:33 * L]
                nc.vector.tensor_tensor_scan(
                    x3[:], dsc[0:16, :], s2seg,
                    initial=x3[:, L - 1:L], op0=ALU.mult, op1=ALU.add)
                nc.scalar.mul(o3[:, 0:1], y3[:, L:L + 1], CS)
                nc.vector.tensor_tensor_scan(
                    y3[:, 1:L + 1], x3[:], dsc[0:16, :],
                    initial=y3[:, L:L + 1], op0=ALU.add, op1=ALU.mult)
                nc.scalar.mul(o3[:, 1:L], y3[:, 1:L], CS)
                nc.sync.dma_start(out=out_d[:, kk * L:(kk + 1) * L], in_=o3[:])

        produce_L1(0)
        for k in range(NB + LAG):
            if k + 1 < NB:
                produce_L1(k + 1)
            if k >= LAG:
                produce_L2(k)
            step_block(k, narrow=(k >= NB))
            post_block(k)

    return _install_wait_split(nc)


def host_prep(spikeInput, W1, W2, core):
    b0 = core * B_LOC
    xs = np.ascontiguousarray(
        spikeInput[b0:b0 + B_LOC].transpose(1, 0, 2)).astype(np.float32)
    w1t = np.ascontiguousarray(W1.T).astype(np.float32)
    w2t = np.empty((128, HC * NOUT), np.float32)
    for hcc in range(HC):
        for o in range(NOUT):
            w2t[:, hcc * NOUT + o] = W2[o, hcc * 128:(hcc + 1) * 128]
    return {"x": xs, "w1t": w1t, "w2t": w2t}


def _get_nc():
    if "nc" not in _nc_cache:
        _nc_cache["nc"] = build()
    return _nc_cache["nc"]


def kernel(spikeInput=None, W1=None, W2=None, _trace=False, **kw):
    spikeInput = np.asarray(spikeInput, dtype=np.float32)
    W1 = np.asarray(W1, dtype=np.float32)
    W2 = np.asarray(W2, dtype=np.float32)
    nc = _get_nc()
    in_maps = [host_prep(spikeInput, W1, W2, c) for c in range(N_CORES)]
    res = run_bass_kernel_spmd(nc, in_maps, list(range(N_CORES)), trace=_trace)
    out = np.empty((B, NOUT, T_FULL), np.float32)
    for c in range(N_CORES):
        o = res.results[c]["out"].reshape(B_LOC, NOUT, T_FULL)
        out[c * B_LOC:(c + 1) * B_LOC] = o
    if _trace:
        return out, res
    return out



# revision 23
# speedup vs baseline: 1.3151x; 1.3151x over previous
"""Self-contained Trainium2 Bass kernel for the SLAYER SNN problem.

kernel(**inputs) takes FULL inputs {spikeInput:[64,4,2000], W1:[512,4],
W2:[2,512]} and returns the FULL [64,2,2000] output. Batch is sharded
8-ways across NeuronCores; each core runs an identical program on its
8 samples.

v2: filter-first production (the psp filter commutes with the channel
mixing, so the input is filtered once on 32 rows instead of per-neuron;
the W1 matmul then emits the filtered drive directly), PSUM eviction fused
with the threshold affine on ACT, and an exact 3-op z-form spike loop:
with states (xr, n=e*yr) and dr=e^-1 the recurrence collapses to
  s = (n <= (P - theta)/20);  xr = dr*xr + s;  n = dr*n + xr.
"""
from contextlib import ExitStack

import numpy as np

import concourse.bass as bass
import concourse.mybir as mybir
from concourse.bass_utils import run_bass_kernel_spmd
from concourse.tile import TileContext
import concourse.tile as _tile_mod
from concourse.vector_clock import ScopedClock as _ScopedClock, VectorClock as _VectorClock


def _drain_and_barrier_split(self, tick_clock, wait_clock):
    # Workaround for walrus "Too many sync wait commands" on the Tile tail
    # drain: emit one drain per processor instead of one multi-wait drain.
    gc = tick_clock.global_clock
    ticks = list(gc)
    for p, t in enumerate(ticks):
        if t <= 0:
            continue
        sub = [t if q == p else 0 for q in range(len(ticks))]
        drain_inst = self.nc.sync.drain()
        wait_clock.add_sem_waits(
            drain_inst.ins, _ScopedClock({None: _VectorClock(sub)}))
    self.nc.all_engine_barrier()
    assert self.sems is not None
    popped = self.nc._tile_sem_poison_stack.pop()
    assert popped is self._sem_poison
    self.nc.clear_and_free_semaphores(list(self.sems.allocated().values()))
    self.nc.all_engine_barrier()


_tile_mod.TileContext._drain_and_barrier = _drain_and_barrier_split


def _split_waits_json(raw):
    # walrus in this container accepts at most one sem-wait per instruction;
    # spill extras onto same-engine Drain carriers placed just before.
    import json as _json
    m = _json.loads(raw)
    ctr = 0
    for fn in m["functions"]:
        for bb in fn["blocks"]:
            out = []
            for i in bb.get("instructions", []):
                si = i.get("sync_info") or {}
                w = si.get("on_wait") or []
                if len(w) > 1:
                    for chunk in w[:-1]:
                        ctr += 1
                        out.append({
                            "debug": i.get("debug", 0), "engine": i["engine"],
                            "ins": [], "name": f"I-WS{ctr}", "opcode": "Drain",
                            "outs": [], "sync_info": {"on_wait": [chunk]},
                        })
                    si = dict(si)
                    si["on_wait"] = w[-1:]
                    i = dict(i)
                    i["sync_info"] = si
                out.append(i)
            bb["instructions"] = out
    return _json.dumps(m).encode()


def _install_wait_split(nc):
    orig = nc.to_json_bytes
    nc.to_json_bytes = lambda: _split_waits_json(orig())
    return nc

F32 = mybir.dt.float32
ALU = mybir.AluOpType

DS = float(np.exp(np.float32(-1.0 / 10.0), dtype=np.float32))
DR = float(np.exp(np.float32(-1.0 / 1.0), dtype=np.float32))
CS = float(np.float32(np.e / 10.0))
TH = 10.0

B = 64
N_CORES = 8
B_LOC = 8
NIN = 4
H = 512
HC = 4
NOUT = 2
NJ = 34
PAD_B = 128
LAG = 2
T_FULL = 2000
L_BLK = 125

_nc_cache = {}


def build(T: int = T_FULL, L: int = L_BLK):
    NB = T // L
    assert NB * L == T and L <= 125 and NB >= LAG
    nc = bass.Bass("TRN2", target_bir_lowering=False, debug=False, num_devices=N_CORES)

    # x rows are (b*4 + i): 32 independent psp-filter lanes
    x_in = nc.declare_dram_parameter("x", [4 * B_LOC, T], F32, isOutput=False)
    w1t_in = nc.declare_dram_parameter("w1t", [NIN, H], F32, isOutput=False)
    w2t_in = nc.declare_dram_parameter("w2t", [128, HC * NOUT], F32, isOutput=False)
    out_d = nc.declare_dram_parameter("out", [B_LOC * NOUT, T], F32, isOutput=True)

    with TileContext(nc) as tc, ExitStack() as ctx:
        pool = ctx.enter_context(tc.tile_pool(name="main", bufs=1))
        psum = ctx.enter_context(tc.tile_pool(name="ps", bufs=1, space="PSUM"))

        w1t = pool.tile([NIN, H], F32, tag="w1t", name="w1t")
        nc.sync.dma_start(out=w1t[:], in_=w1t_in[:])
        w2t = pool.tile([128, HC * NOUT], F32, tag="w2t", name="w2t")
        nc.sync.dma_start(out=w2t[:], in_=w2t_in[:])

        dsc32 = pool.tile([32, T], F32, tag="dsc32", name="dsc32")
        nc.vector.memset(dsc32[:], DS)
        dsc16 = pool.tile([16, L], F32, tag="dsc16", name="dsc16")
        nc.vector.memset(dsc16[:], DS)
        biasb = pool.tile([128, 1], F32, tag="biasb", name="biasb")
        nc.vector.memset(biasb[:], -TH / 20.0)

        def zeros(shape, tag, dtype=F32, eng=None):
            t = pool.tile(shape, dtype, tag=tag, name=tag)
            (eng or nc.vector).memset(t[:], 0.0)
            return t

        # ---- input staging + psp filter (filter-first) ----
        x_t = pool.tile([32, T], F32, tag="xt", name="xt")
        nc.sync.dma_start(out=x_t[:], in_=x_in[:])
        xsf = pool.tile([32, T], F32, tag="xsf", name="xsf")
        F1 = pool.tile([32, T], F32, tag="F1", name="F1")  # ys_after(t)
        nc.vector.tensor_tensor_scan(
            xsf[:], dsc32[:], x_t[:],
            initial=0.0, op0=ALU.mult, op1=ALU.add)
        nc.vector.tensor_tensor_scan(
            F1[:], xsf[:], dsc32[:],
            initial=0.0, op0=ALU.add, op1=ALU.mult)
        # repack to 4 partitions (b into free dim) for the PE contraction:
        # F1f[i, b*(T+1) + t + 1] = ys_after(b,i,t); guard col per b = 0 so
        # the rhs slice [t0 .. t0+L) reads ys_after(t-1) = P(t)/cs.
        F1f = zeros([NIN, B_LOC * (T + 1)], "F1f")
        for b in range(B_LOC):
            nc.sync.dma_start(
                out=F1f[:, b * (T + 1) + 1:(b + 1) * (T + 1)],
                in_=F1[4 * b:4 * b + 4, :])

        # ---- spike-loop state ----
        n_t = zeros([128, NJ], "n")      # n = e * yr  (refractory, scaled)
        xr = zeros([128, NJ], "xr")

        TPn_blk = [zeros([128, NJ * L], f"P{i}") for i in range(2)]
        S_blk = [zeros([128, NJ * L], f"S{i}") for i in range(2)]

        # ---- layer-2 drive staging ----
        x2f = zeros([16, L], "x2")
        F2 = zeros([16, L + 1], "F2")    # col 0 carries ys_after from prev blk
        a2_ps = [psum.tile([2, B_LOC * PAD_B], F32, tag=f"a2ps{i}", name=f"a2ps{i}")
                 for i in range(2)]
        a2_sb = [pool.tile([2, B_LOC * L], F32, tag=f"a2sb{i}", name=f"a2sb{i}")
                 for i in range(2)]
        a2_16 = [pool.tile([16, L], F32, tag=f"a216{i}", name=f"a216{i}")
                 for i in range(2)]
        x3 = zeros([16, L], "x3")
        y3 = zeros([16, L + 1], "y3")
        o3 = pool.tile([16, L], F32, tag="o3", name="o3")
        a1_ps = [psum.tile([128, L], F32, tag=f"a1ps{i}", name=f"a1ps{i}")
                 for i in range(3)]

        def produce_L1(k):
            # TPn for block k: matmul of pre-filtered input, then ACT
            # eviction fused with the (P - theta)/20 affine.
            Pb = TPn_blk[k % 2]
            t0 = k * L
            for j in range(32):
                hc, b = divmod(j, B_LOC)
                aps = a1_ps[j % 3]
                nc.tensor.matmul(
                    aps[:],
                    lhsT=w1t[:, hc * 128:(hc + 1) * 128],
                    rhs=F1f[:, b * (T + 1) + t0:b * (T + 1) + t0 + L],
                    start=True, stop=True)
                nc.scalar.activation(
                    Pb[:, j * L:(j + 1) * L], aps[:],
                    mybir.ActivationFunctionType.Identity,
                    bias=biasb[:], scale=CS / 20.0)

        def produce_L2(k):
            # TPn for the L2 column of block k-LAG: psp-filter a2 then affine.
            Pb = TPn_blk[k % 2]
            asb2 = a2_sb[k % 2]
            a16 = a2_16[k % 2]
            nc.scalar.copy(
                asb2[:].rearrange("o (b t) -> o b t", t=L),
                a2_ps[(k - LAG) % 2][:].rearrange("o (b t) -> o b t", t=PAD_B)[:, :, 0:L])
            for b in range(B_LOC):
                # pack p = 2*b + o (DMA partition shift; contiguous 2-row dst)
                nc.sync.dma_start(out=a16[2 * b:2 * b + 2, :],
                                  in_=asb2[:, b * L:(b + 1) * L])
            nc.vector.tensor_tensor_scan(
                x2f[:], dsc16[:], a16[:],
                initial=x2f[:, L - 1:L], op0=ALU.mult, op1=ALU.add)
            nc.vector.tensor_tensor_scan(
                F2[:, 1:L + 1], x2f[:], dsc16[:],
                initial=F2[:, L:L + 1], op0=ALU.add, op1=ALU.mult)
            nc.gpsimd.tensor_scalar(
                out=Pb[0:16, 32 * L:33 * L], in0=F2[:, 0:L],
                scalar1=CS / 20.0, scalar2=TH / 20.0,
                op0=ALU.mult, op1=ALU.subtract)
            nc.scalar.copy(F2[:, 0:1], F2[:, L:L + 1])

        def step_block(k, narrow=False):
            Pb, Sb = TPn_blk[k % 2], S_blk[k % 2]
            for tau in range(L):
                if narrow:
                    # only the L2 column is live (tail blocks)
                    Pcol = Pb[0:16, 32 * L + tau:32 * L + tau + 1]
                    Scol = Sb[0:16, 32 * L + tau:32 * L + tau + 1]
                    nn, xx = n_t[0:16, 32:33], xr[0:16, 32:33]
                else:
                    Pcol, Scol = Pb[:, tau::L], Sb[:, tau::L]
                    nn, xx = n_t[:], xr[:]
                nc.vector.tensor_tensor(out=Scol, in0=nn, in1=Pcol, op=ALU.is_le)
                nc.vector.scalar_tensor_tensor(xx, xx, DR, Scol, ALU.mult, ALU.add)
                nc.vector.scalar_tensor_tensor(nn, nn, DR, xx, ALU.mult, ALU.add)

        def post_block(k):
            Sb = S_blk[k % 2]
            if k < NB:
                for b in range(B_LOC):
                    for hc in range(HC):
                        j = hc * B_LOC + b
                        nc.tensor.matmul(
                            a2_ps[k % 2][:, b * PAD_B:b * PAD_B + L],
                            lhsT=w2t[:, hc * NOUT:(hc + 1) * NOUT],
                            rhs=Sb[:, j * L:(j + 1) * L],
                            start=(hc == 0), stop=(hc == HC - 1))
            if k >= LAG:
                kk = k - LAG
                s2seg = Sb[0:16, 32 * L:33 * L]
                nc.vector.tensor_tensor_scan(
                    x3[:], dsc16[:], s2seg,
                    initial=x3[:, L - 1:L], op0=ALU.mult, op1=ALU.add)
                nc.scalar.mul(o3[:, 0:1], y3[:, L:L + 1], CS)
                nc.vector.tensor_tensor_scan(
                    y3[:, 1:L + 1], x3[:], dsc16[:],
                    initial=y3[:, L:L + 1], op0=ALU.add, op1=ALU.mult)
                nc.scalar.mul(o3[:, 1:L], y3[:, 1:L], CS)
                nc.sync.dma_start(out=out_d[:, kk * L:(kk + 1) * L], in_=o3[:])

        produce_L1(0)
        for k in range(NB + LAG):
            if k + 1 < NB:
                produce_L1(k + 1)
            if k >= LAG:
                produce_L2(k)
            step_block(k, narrow=(k >= NB))
            post_block(k)

    return _install_wait_split(nc)


def host_prep(spikeInput, W1, W2, core):
    b0 = core * B_LOC
    # rows (b*4 + i)
    xs = np.ascontiguousarray(
        spikeInput[b0:b0 + B_LOC].reshape(4 * B_LOC, T_FULL)).astype(np.float32)
    w1t = np.ascontiguousarray(W1.T).astype(np.float32)
    w2t = np.empty((128, HC * NOUT), np.float32)
    for hcc in range(HC):
        for o in range(NOUT):
            w2t[:, hcc * NOUT + o] = W2[o, hcc * 128:(hcc + 1) * 128]
    return {"x": xs, "w1t": w1t, "w2t": w2t}


def _get_nc():
    if "nc" not in _nc_cache:
        _nc_cache["nc"] = build()
    return _nc_cache["nc"]


def kernel(spikeInput=None, W1=None, W2=None, _trace=False, **kw):
    spikeInput = np.asarray(spikeInput, dtype=np.float32)
    W1 = np.asarray(W1, dtype=np.float32)
    W2 = np.asarray(W2, dtype=np.float32)
    nc = _get_nc()
    in_maps = [host_prep(spikeInput, W1, W2, c) for c in range(N_CORES)]
    res = run_bass_kernel_spmd(nc, in_maps, list(range(N_CORES)), trace=_trace)
    out = np.empty((B, NOUT, T_FULL), np.float32)
    for c in range(N_CORES):
        o = res.results[c]["out"].reshape(B_LOC, NOUT, T_FULL)
        out[c * B_LOC:(c + 1) * B_LOC] = o
    if _trace:
        return out, res
    return out
